# revision 1
# baseline (speedup 1.0000x reference)
"""Trainium2 Bass kernel for Disk descriptor mutual-NN matching (retrieval_knn).

Strategy (8 NeuronCores, shard descriptors1 columns M across cores):
  - Each core c holds full d0 [256, 8192] and its d1 shard [256, 1024].
  - Forward:  S_c = d0.T @ d1_c  (fp32 PE matmul) -> per-row (N) top-8 values
    + indices over the core's 1024 local columns via the DVE top-8
    instruction (InstMax / InstMaxIndex).  Host merges per-core top-2s.
  - Backward: instead of recomputing S^T, PE-transposes the forward S tiles
    (exact bit-preserving data movement) and reduces columns via staged
    InstMax top-8 merges.  Only column top-2 VALUES are needed: the mutual
    check `bck_nn[fwd_nn[i]] == i` is replaced by the exactly-equivalent
    value test `S[i,j] == colmax[j]` (ties in either formulation force the
    backward ratio test to fail identically, so outputs match bit-for-bit).
  - Host applies the exact reference arithmetic in float32 (sqrt transform,
    division-based ratio test, tie semantics).
"""

import sys

if "/opt/trn_rl_repo" not in sys.path:
    sys.path.insert(0, "/opt/trn_rl_repo")

import numpy as np

N_KPTS = 8192
M_KPTS = 8192
F_DIM = 256
N_CORES = 8
M_SHARD = M_KPTS // N_CORES  # 1024

SQRT_2 = np.float32(1.414213)
CLIP_LO = np.float32(1e-6)
ONE = np.float32(1.0)

GROUP = 4  # fwd row-chunks per transpose group

# fp32: native fp32 matmuls (4 cyc/row).  f16x3: split each f32 input into
# f16 high part + scaled f16 residual; S = h0*h1 + h0s*l1s + l0s*h1s
# accumulated in one PSUM group (3 f16 matmuls at 1 cyc/row each = 3/4 the
# PE time).  All products are exact or quantized below fp32 accumulation
# noise, so ranking quality matches native fp32.
DTYPE_MODE = "f16x3"


def _split_f16(a32):
    """f32 -> (h, h/32, 32*(a-h)) as float16, with f16-subnormal highs
    flushed into the residual so no information rides on f16 subnormals."""
    h = a32.astype(np.float16)
    h[np.abs(a32) < 6.104e-5] = np.float16(0)
    l = a32 - h.astype(np.float32)
    h_s = (h.astype(np.float32) / 32.0).astype(np.float16)
    l_s = (l * 32.0).astype(np.float16)
    return h, h_s, l_s


# --------------------------------------------------------------------------
# Device kernel builder
# --------------------------------------------------------------------------

def build_kernel(n_rows=N_KPTS, m_shard=M_SHARD, f_dim=F_DIM, repeat=1,
                 dtype_mode=DTYPE_MODE):
    """Build the per-core SPMD Bass program.

    Inputs (per core):
      d0: [kf, 128, n_rows] f32   (descriptors0, K-chunked)
      d1: [kf, 128, m_shard] f32  (this core's descriptors1 shard)
    Outputs (per core):
      fwd_val [128, n_chunks*8] f32, fwd_idx [128, n_chunks*8] u32
      bwd_val [128, m_chunks*8] f32   (column top-8 values, rows = local col)
    """
    import concourse.bacc as bacc
    import concourse.mybir as mybir
    import concourse.tile as tile
    from concourse.masks import make_identity

    kf = f_dim // 128
    n_chunks = n_rows // 128          # forward row chunks
    m_tiles = max(1, m_shard // 512)  # 512-wide column tiles per fwd chunk
    mw = min(512, m_shard)
    m_chunks = m_shard // 128         # backward column chunks
    assert n_chunks % GROUP == 0 and m_shard % 128 == 0 and f_dim % 128 == 0
    n_groups = n_chunks // GROUP
    pair = 2 if n_groups % 2 == 0 else 1    # transpose groups per staging
    n_pairs = n_groups // pair
    n_stages = n_pairs

    nc = bacc.Bacc("TRN2", target_bir_lowering=False, debug=False,
                   num_devices=1)

    if dtype_mode == "fp32":
        in_names = ["d0", "d1"]
        in_dt = mybir.dt.float32
    else:
        in_names = ["d0h", "d0hs", "d0ls", "d1h", "d1hs", "d1ls"]
        in_dt = mybir.dt.float16
    in_dram = {}
    for nm in in_names:
        nw = n_rows if nm.startswith("d0") else m_shard
        in_dram[nm] = nc.dram_tensor(nm, [kf, 128, nw], in_dt,
                                     kind="ExternalInput")
    fwd_val = nc.dram_tensor("fwd_val", [128, n_chunks * 8], mybir.dt.float32,
                             kind="ExternalOutput")
    fwd_idx = nc.dram_tensor("fwd_idx", [128, n_chunks * 8], mybir.dt.uint32,
                             kind="ExternalOutput")
    bwd_val = nc.dram_tensor("bwd_val", [128, m_chunks * 8], mybir.dt.float32,
                             kind="ExternalOutput")

    with tile.TileContext(nc) as tc:
        with tc.tile_pool(name="persist", bufs=1) as persist, \
             tc.tile_pool(name="schunk", bufs=pair * GROUP + 2) as schunk_pool, \
             tc.tile_pool(name="stg", bufs=3) as stg_pool, \
             tc.tile_pool(name="outs", bufs=1) as outs_pool, \
             tc.tile_pool(name="psf", bufs=2, space="PSUM") as psf, \
             tc.tile_pool(name="ptp", bufs=2, space="PSUM") as ptp:

            # resident inputs; d0 loads split along n so early fwd units
            # unblock before the full load completes
            in_sb = {}
            for nm in in_names:
                nw = n_rows if nm.startswith("d0") else m_shard
                in_sb[nm] = [persist.tile([128, nw], in_dt,
                                          name=f"{nm}sb{k}", tag=f"{nm}sb{k}")
                             for k in range(kf)]
            # interleave d1 loads with the first d0 piece so the critical
            # first-unit inputs land on distinct DMA queues immediately
            n_split = 8 if n_rows % 1024 == 0 else 1
            d0n = [nm for nm in in_names if nm.startswith("d0")]
            d1n = [nm for nm in in_names if nm.startswith("d1")]
            first = []
            for k in range(kf):
                for i in range(max(len(d0n), len(d1n))):
                    if i < len(d1n):
                        first.append((d1n[i], k, None))
                    if i < len(d0n):
                        first.append((d0n[i], k, 0))
            for nm, k, p in first:
                if p is None:
                    nc.sync.dma_start(in_sb[nm][k][:], in_dram[nm][k])
                else:
                    sl = slice(0, n_rows // n_split)
                    nc.sync.dma_start(in_sb[nm][k][:, sl],
                                      in_dram[nm][k][:, sl])
            for p in range(1, n_split):
                sl = slice(p * n_rows // n_split, (p + 1) * n_rows // n_split)
                for nm in d0n:
                    for k in range(kf):
                        nc.sync.dma_start(in_sb[nm][k][:, sl],
                                          in_dram[nm][k][:, sl])
            if dtype_mode == "fp32":
                # (lhsT source, rhs source) per accumulation term
                terms = [("d0", "d1")]
            else:
                terms = [("d0h", "d1h"), ("d0hs", "d1ls"), ("d0ls", "d1hs")]


            ident = persist.tile([128, 128], mybir.dt.float32, name="ident")
            make_identity(nc, ident[:])
            # warm-up matmul: starts the PE p-state ramp clock while input
            # DMAs are still streaming (identity needs no DMA)
            warm = psf.tile([128, 8], mybir.dt.float32, tag="pf", name="warm",
                            padded_shape=[128, m_tiles * mw])
            nc.tensor.matmul(warm[:], ident[:], ident[:, :8],
                             start=True, stop=True)

            fv_sb = outs_pool.tile([128, n_chunks * 8], mybir.dt.float32)
            fi_sb = outs_pool.tile([128, n_chunks * 8], mybir.dt.uint32)
            bv_sb = outs_pool.tile([128, m_chunks * 8], mybir.dt.float32)
            # per-mm candidate buffers (top-8 of each staging window)
            cand = [outs_pool.tile([128, max(8, n_stages * 8)],
                                   mybir.dt.float32, name=f"cand{mm}",
                                   tag=f"cand{mm}")
                    for mm in range(m_chunks)]

            for _rep in range(repeat):
                def fwd_unit(n):
                    s_chunk = schunk_pool.tile([128, m_shard],
                                               mybir.dt.float32, tag="schunk")
                    # one PSUM tile spanning m_tiles banks; each matmul
                    # writes within a single bank; one wide ACT copy drains
                    pf = psf.tile([128, m_tiles * mw], mybir.dt.float32,
                                  tag="pf", name="pf")
                    n_acc = kf * len(terms)
                    for k in range(kf):
                        for ti, (lnm, rnm) in enumerate(terms):
                            # weight (lhsT) loaded once, reused across m
                            for m in range(m_tiles):
                                acc = k * len(terms) + ti
                                nc.tensor.matmul(
                                    pf[:, m * mw:(m + 1) * mw],
                                    in_sb[lnm][k][:, n * 128:(n + 1) * 128],
                                    in_sb[rnm][k][:, m * mw:(m + 1) * mw],
                                    start=(acc == 0), stop=(acc == n_acc - 1))
                    nc.scalar.copy(s_chunk[:], pf[:])
                    nc.vector.max(out=fv_sb[:, n * 8:(n + 1) * 8],
                                  in_=s_chunk[:])
                    nc.vector.max_index(out=fi_sb[:, n * 8:(n + 1) * 8],
                                        in_max=fv_sb[:, n * 8:(n + 1) * 8],
                                        in_values=s_chunk[:])
                    return s_chunk

                wpp = pair * GROUP * 128  # rows covered per group-pair
                for gp in range(n_pairs):
                    chunks = [fwd_unit(gp * pair * GROUP + j)
                              for j in range(pair * GROUP)]
                    for mm in range(m_chunks):
                        pt = ptp.tile([128, wpp], mybir.dt.float32, tag="pt")
                        for j in range(pair * GROUP):
                            nc.tensor.transpose(
                                pt[:, j * 128:(j + 1) * 128],
                                chunks[j][:, mm * 128:(mm + 1) * 128],
                                ident[:])
                        stg = stg_pool.tile([128, wpp], mybir.dt.float32,
                                            name="stg", tag="stg")
                        nc.scalar.copy(stg[:], pt[:])
                        nc.vector.max(out=cand[mm][:, gp * 8:(gp + 1) * 8],
                                      in_=stg[:])
                for mm in range(m_chunks):
                    if n_pairs > 1:
                        nc.vector.max(out=bv_sb[:, mm * 8:(mm + 1) * 8],
                                      in_=cand[mm][:])
                    else:
                        nc.vector.tensor_copy(bv_sb[:, mm * 8:(mm + 1) * 8],
                                              cand[mm][:, :8])

            # stream forward outputs out as they complete (shorter tail)
            ow = n_chunks * 8 // max(1, min(4, n_pairs))
            for p in range(n_chunks * 8 // ow):
                sl = slice(p * ow, (p + 1) * ow)
                nc.sync.dma_start(fwd_val[:, sl], fv_sb[:, sl])
                nc.sync.dma_start(fwd_idx[:, sl], fi_sb[:, sl])
            nc.sync.dma_start(bwd_val[:], bv_sb[:])

    nc.compile()
    return nc


_KERNEL_CACHE = {}


def get_kernel(repeat=1, dtype_mode=DTYPE_MODE):
    key = (repeat, dtype_mode)
    if key not in _KERNEL_CACHE:
        _KERNEL_CACHE[key] = build_kernel(repeat=repeat,
                                          dtype_mode=dtype_mode)
    return _KERNEL_CACHE[key]


# --------------------------------------------------------------------------
# Host side
# --------------------------------------------------------------------------

def _decode_top8(arr, chunks):
    """[128, chunks*8] -> [chunks*128, 8] with row r = chunk*128 + partition."""
    return arr.reshape(128, chunks, 8).transpose(1, 0, 2).reshape(chunks * 128, 8)


def run_device(descriptors0, descriptors1, repeat=1, dtype_mode=DTYPE_MODE):
    """Run the SPMD kernel on 8 cores. Returns per-core raw outputs."""
    from concourse.bass_utils import run_bass_kernel_spmd

    nc = get_kernel(repeat, dtype_mode)
    d0 = np.ascontiguousarray(descriptors0[0]).astype(np.float32, copy=False)
    d1 = np.ascontiguousarray(descriptors1[0]).astype(np.float32, copy=False)
    kf = F_DIM // 128

    def shard(a, c):
        return np.ascontiguousarray(
            a[:, c * M_SHARD:(c + 1) * M_SHARD]).reshape(kf, 128, M_SHARD)

    if dtype_mode == "fp32":
        d0r = d0.reshape(kf, 128, N_KPTS)
        in_maps = [{"d0": d0r, "d1": shard(d1, c)} for c in range(N_CORES)]
    else:
        h0, h0s, l0s = _split_f16(d0)
        h1, h1s, l1s = _split_f16(d1)
        d0m = {"d0h": h0.reshape(kf, 128, N_KPTS),
               "d0hs": h0s.reshape(kf, 128, N_KPTS),
               "d0ls": l0s.reshape(kf, 128, N_KPTS)}
        in_maps = [dict(d0m, d1h=shard(h1, c), d1hs=shard(h1s, c),
                        d1ls=shard(l1s, c)) for c in range(N_CORES)]
    last_err = None
    for _attempt in range(3):
        try:
            res = run_bass_kernel_spmd(nc, in_maps, list(range(N_CORES)))
            return res.results
        except Exception as e:  # rare transient device-unrecoverable flakes
            last_err = e
    raise last_err


def postprocess(results):
    """Merge per-core device outputs into the reference's 4 output arrays."""
    n = N_KPTS
    n_chunks = n // 128
    m_chunks = M_SHARD // 128

    # ---- forward: merge per-core top-2 into global top-2 ----
    m1 = np.empty((N_CORES, n), np.float32)
    m2 = np.empty((N_CORES, n), np.float32)
    i1 = np.empty((N_CORES, n), np.int64)
    for c in range(N_CORES):
        vals = _decode_top8(results[c]["fwd_val"], n_chunks)
        idxs = _decode_top8(results[c]["fwd_idx"], n_chunks)
        m1[c] = vals[:, 0]
        m2[c] = vals[:, 1]
        i1[c] = idxs[:, 0].astype(np.int64) + c * M_SHARD

    w = np.argmax(m1, axis=0)                      # first max on ties
    rows = np.arange(n)
    s1 = m1[w, rows]
    fwd_nn = i1[w, rows]
    m1_masked = m1.copy()
    m1_masked[w, rows] = -np.inf
    s2 = np.maximum(m1_masked.max(axis=0), m2[w, rows]).astype(np.float32)

    # ---- backward: concatenate per-core full-column top-2 values ----
    cm1 = np.empty(M_KPTS, np.float32)
    cm2 = np.empty(M_KPTS, np.float32)
    for c in range(N_CORES):
        vals = _decode_top8(results[c]["bwd_val"], m_chunks)
        sl = slice(c * M_SHARD, (c + 1) * M_SHARD)
        cm1[sl] = vals[:, 0]
        cm2[sl] = vals[:, 1]

    # ---- exact reference arithmetic (float32) ----
    def dist(s):
        return SQRT_2 * np.sqrt(np.maximum(ONE - s, CLIP_LO))

    fd1, fd2 = dist(s1), dist(s2)
    fwd_ok = (fd1 / fd2) < np.float32(1.0)
    bd1, bd2 = dist(cm1), dist(cm2)
    bck_ok = (bd1 / bd2) < np.float32(1.0)

    # mutual NN: row i's best value must BE column j's max (bitwise; exact
    # because the backward path transposes the very same f32 tiles).  Ties
    # where this differs from index-equality are exactly the cases where
    # bck_ok / fwd_ok are False in both formulations.
    mutual = fwd_ok & bck_ok[fwd_nn] & (s1 == cm1[fwd_nn])

    indices0 = np.where(mutual, fwd_nn, -1)[None, :].astype(np.int32)
    mscores0 = (indices0 > 0).astype(np.int32)
    matches1 = np.full((1, M_KPTS), -1, dtype=np.int32)
    mscores1 = np.zeros((1, M_KPTS), dtype=np.float32)
    return indices0, matches1, mscores0, mscores1


def kernel(descriptors0, descriptors1, keypoints0, keypoints1):
    results = run_device(descriptors0, descriptors1)
    return postprocess(results)



# revision 9
# speedup vs baseline: 1.2826x; 1.2826x over previous
"""Trainium2 Bass kernel for Disk descriptor mutual-NN matching (retrieval_knn).

Strategy (8 NeuronCores, shard descriptors1 columns M across cores):
  - Each core c holds full d0 [256, 8192] and its d1 shard [256, 1024].
  - S_c = d0.T @ d1_c via f16x3 split matmuls (3 f16 passes = 3/4 the fp32
    PE time; products exact below fp32 accumulation noise).
  - Forward: per-row top-8 values over the core's 1024 columns (DVE
    InstMax).  No index pass: indices are recovered on the host (below).
  - Backward: per-lane running elementwise max over the 64 row-chunks
    (DVE tensor_max fold, 1 op/chunk) -> fold[p, j] = max over rows
    {p + 128k} of column j.  8 small PE transposes of the fold state +
    DVE top-8 give the exact column max M1[j] (bit-identical to the
    forward S values) and a cross-lane runner-up M2x[j] (== true second
    unless the column's top-2 share a lane, which only matters on exact
    value ties -- where the reference fails the ratio test anyway; with
    rt=1.0 the ratio test is a pure tie detector).
  - Host mutual test: row i is matched to column j iff its top value
    s1[i] IS the column max: s1[i] == M1[j], exact because both sides
    are the same f32 bits from the same PSUM drain.  This also recovers
    the matched column INDEX by value search (unique absent f32 bit
    collisions between distinct dot products, which are detected).
"""

import sys

if "/opt/trn_rl_repo" not in sys.path:
    sys.path.insert(0, "/opt/trn_rl_repo")

import numpy as np

N_KPTS = 8192
M_KPTS = 8192
F_DIM = 256
N_CORES = 8
M_SHARD = M_KPTS // N_CORES  # 1024

SQRT_2 = np.float32(1.414213)
CLIP_LO = np.float32(1e-6)
ONE = np.float32(1.0)

DTYPE_MODE = "f16x3"


def _split_f16(a32):
    """f32 -> (h, h/32, 32*(a-h)) as float16, with f16-subnormal highs
    flushed into the residual so no information rides on f16 subnormals."""
    h = a32.astype(np.float16)
    h[np.abs(a32) < 6.104e-5] = np.float16(0)
    l = a32 - h.astype(np.float32)
    h_s = (h.astype(np.float32) / 32.0).astype(np.float16)
    l_s = (l * 32.0).astype(np.float16)
    return h, h_s, l_s


# --------------------------------------------------------------------------
# Device kernel builder
# --------------------------------------------------------------------------

def build_kernel(n_rows=N_KPTS, m_shard=M_SHARD, f_dim=F_DIM,
                 dtype_mode=DTYPE_MODE):
    """Build the per-core SPMD Bass program.

    Inputs (per core, f16 split terms):
      d0h/d0hs/d0ls: [kf, 128, n_rows]   (descriptors0, K-chunked)
      d1h/d1hs/d1ls: [kf, 128, m_shard]  (this core's descriptors1 shard)
    Outputs (per core):
      fwd_val [128, n_chunks*8] f32  (row top-8 per 128-row chunk)
      bwd_val [128, m_chunks*8] f32  (top-8 of per-lane column maxes;
                                      col 0 = exact column max)
    """
    import concourse.bacc as bacc
    import concourse.mybir as mybir
    import concourse.tile as tile
    from concourse.masks import make_identity

    kf = f_dim // 128
    n_chunks = n_rows // 128          # row chunks
    m_tiles = max(1, m_shard // 512)  # 512-wide column tiles (PSUM banks)
    mw = min(512, m_shard)
    m_chunks = m_shard // 128         # backward column chunks
    assert m_shard % 128 == 0 and f_dim % 128 == 0

    nc = bacc.Bacc("TRN2", target_bir_lowering=False, debug=False,
                   num_devices=1)

    if dtype_mode == "fp32":
        in_names = ["d0", "d1"]
        in_dt = mybir.dt.float32
    else:
        in_names = ["d0h", "d0hs", "d0ls", "d1h", "d1hs", "d1ls"]
        in_dt = mybir.dt.float16
    in_dram = {}
    for nm in in_names:
        nw = n_rows if nm.startswith("d0") else m_shard
        in_dram[nm] = nc.dram_tensor(nm, [kf, 128, nw], in_dt,
                                     kind="ExternalInput")
    fwd_val = nc.dram_tensor("fwd_val", [128, n_chunks * 8], mybir.dt.float32,
                             kind="ExternalOutput")
    bwd_val = nc.dram_tensor("bwd_val", [128, m_chunks * 8], mybir.dt.float32,
                             kind="ExternalOutput")
    bwd_lane = nc.dram_tensor("bwd_lane", [128, m_chunks * 8],
                              mybir.dt.uint32, kind="ExternalOutput")

    with tile.TileContext(nc) as tc:
        with tc.tile_pool(name="persist", bufs=1) as persist, \
             tc.tile_pool(name="schunk", bufs=4) as schunk_pool, \
             tc.tile_pool(name="outs", bufs=1) as outs_pool, \
             tc.tile_pool(name="psf", bufs=2, space="PSUM") as psf, \
             tc.tile_pool(name="ptp", bufs=1, space="PSUM") as ptp:

            # resident inputs; d0 loads split along n so early fwd units
            # unblock before the full load completes
            in_sb = {}
            for nm in in_names:
                nw = n_rows if nm.startswith("d0") else m_shard
                in_sb[nm] = [persist.tile([128, nw], in_dt,
                                          name=f"{nm}sb{k}", tag=f"{nm}sb{k}")
                             for k in range(kf)]
            # interleave d1 loads with the first d0 piece so the critical
            # first-unit inputs land on distinct DMA queues immediately
            n_split = 8 if n_rows % 1024 == 0 else 1
            d0n = [nm for nm in in_names if nm.startswith("d0")]
            d1n = [nm for nm in in_names if nm.startswith("d1")]
            first = []
            for k in range(kf):
                for i in range(max(len(d0n), len(d1n))):
                    if i < len(d1n):
                        first.append((d1n[i], k, None))
                    if i < len(d0n):
                        first.append((d0n[i], k, 0))
            for nm, k, p in first:
                if p is None:
                    nc.sync.dma_start(in_sb[nm][k][:], in_dram[nm][k])
                else:
                    sl = slice(0, n_rows // n_split)
                    nc.sync.dma_start(in_sb[nm][k][:, sl],
                                      in_dram[nm][k][:, sl])
            for p in range(1, n_split):
                sl = slice(p * n_rows // n_split, (p + 1) * n_rows // n_split)
                for nm in d0n:
                    for k in range(kf):
                        nc.sync.dma_start(in_sb[nm][k][:, sl],
                                          in_dram[nm][k][:, sl])
            if dtype_mode == "fp32":
                terms = [("d0", "d1")]
            else:
                terms = [("d0h", "d1h"), ("d0hs", "d1ls"), ("d0ls", "d1hs")]

            ident = persist.tile([128, 128], mybir.dt.float32, name="ident")
            make_identity(nc, ident[:])
            # warm-up matmul: starts the PE p-state ramp clock while input
            # DMAs are still streaming (identity needs no DMA)
            warm = psf.tile([128, 8], mybir.dt.float32, tag="pf", name="warm",
                            padded_shape=[128, m_tiles * mw])
            nc.tensor.matmul(warm[:], ident[:], ident[:, :8],
                             start=True, stop=True)

            fv_sb = outs_pool.tile([128, n_chunks * 8], mybir.dt.float32)
            bv_sb = outs_pool.tile([128, m_chunks * 8], mybir.dt.float32)
            bl_sb = outs_pool.tile([128, m_chunks * 8], mybir.dt.uint32)
            # per-lane running column max over row chunks
            fold = outs_pool.tile([128, m_shard], mybir.dt.float32,
                                  name="fold")

            n_acc = kf * len(terms)
            for n in range(n_chunks):
                s_chunk = schunk_pool.tile([128, m_shard],
                                           mybir.dt.float32, tag="schunk")
                # one PSUM tile spanning m_tiles banks; each matmul
                # writes within a single bank; one wide ACT copy drains
                pf = psf.tile([128, m_tiles * mw], mybir.dt.float32,
                              tag="pf", name="pf")
                for k in range(kf):
                    for ti, (lnm, rnm) in enumerate(terms):
                        for m in range(m_tiles):
                            acc = k * len(terms) + ti
                            nc.tensor.matmul(
                                pf[:, m * mw:(m + 1) * mw],
                                in_sb[lnm][k][:, n * 128:(n + 1) * 128],
                                in_sb[rnm][k][:, m * mw:(m + 1) * mw],
                                start=(acc == 0), stop=(acc == n_acc - 1))
                nc.scalar.copy(s_chunk[:], pf[:])
                # backward fold first: it is the longest dependence chain
                # (the tail transposes wait on the final fold)
                if n == 0:
                    nc.vector.tensor_copy(fold[:], s_chunk[:])
                else:
                    nc.vector.tensor_max(fold[:], fold[:], s_chunk[:])
                nc.vector.max(out=fv_sb[:, n * 8:(n + 1) * 8],
                              in_=s_chunk[:])
                # stream forward outputs out as they complete
                if (n + 1) % (n_chunks // 4) == 0:
                    sl = slice((n + 1 - n_chunks // 4) * 8, (n + 1) * 8)
                    nc.sync.dma_start(fwd_val[:, sl], fv_sb[:, sl])

            # transpose the fold state; top-8 over lanes per column, plus
            # the winning lane index (disambiguates host value matching)
            pt = ptp.tile([128, m_shard], mybir.dt.float32, tag="pt")
            for mm in range(m_chunks):
                nc.tensor.transpose(pt[:, mm * 128:(mm + 1) * 128],
                                    fold[:, mm * 128:(mm + 1) * 128],
                                    ident[:])
                nc.vector.max(out=bv_sb[:, mm * 8:(mm + 1) * 8],
                              in_=pt[:, mm * 128:(mm + 1) * 128])
                nc.vector.max_index(out=bl_sb[:, mm * 8:(mm + 1) * 8],
                                    in_max=bv_sb[:, mm * 8:(mm + 1) * 8],
                                    in_values=pt[:, mm * 128:(mm + 1) * 128])
            nc.sync.dma_start(bwd_val[:], bv_sb[:])
            nc.sync.dma_start(bwd_lane[:], bl_sb[:])

    nc.compile()
    return nc


_KERNEL_CACHE = {}


def get_kernel(dtype_mode=DTYPE_MODE):
    if dtype_mode not in _KERNEL_CACHE:
        _KERNEL_CACHE[dtype_mode] = build_kernel(dtype_mode=dtype_mode)
    return _KERNEL_CACHE[dtype_mode]


# --------------------------------------------------------------------------
# Host side
# --------------------------------------------------------------------------

def _decode_top8(arr, chunks):
    """[128, chunks*8] -> [chunks*128, 8] with row r = chunk*128 + partition."""
    return arr.reshape(128, chunks, 8).transpose(1, 0, 2).reshape(chunks * 128, 8)


def run_device(descriptors0, descriptors1, dtype_mode=DTYPE_MODE):
    """Run the SPMD kernel on 8 cores. Returns per-core raw outputs."""
    from concourse.bass_utils import run_bass_kernel_spmd

    nc = get_kernel(dtype_mode)
    d0 = np.ascontiguousarray(descriptors0[0]).astype(np.float32, copy=False)
    d1 = np.ascontiguousarray(descriptors1[0]).astype(np.float32, copy=False)
    kf = F_DIM // 128

    def shard(a, c):
        return np.ascontiguousarray(
            a[:, c * M_SHARD:(c + 1) * M_SHARD]).reshape(kf, 128, M_SHARD)

    if dtype_mode == "fp32":
        d0r = d0.reshape(kf, 128, N_KPTS)
        in_maps = [{"d0": d0r, "d1": shard(d1, c)} for c in range(N_CORES)]
    else:
        h0, h0s, l0s = _split_f16(d0)
        h1, h1s, l1s = _split_f16(d1)
        d0m = {"d0h": h0.reshape(kf, 128, N_KPTS),
               "d0hs": h0s.reshape(kf, 128, N_KPTS),
               "d0ls": l0s.reshape(kf, 128, N_KPTS)}
        in_maps = [dict(d0m, d1h=shard(h1, c), d1hs=shard(h1s, c),
                        d1ls=shard(l1s, c)) for c in range(N_CORES)]
    last_err = None
    for _attempt in range(3):
        try:
            res = run_bass_kernel_spmd(nc, in_maps, list(range(N_CORES)))
            # materialize device arrays now so transient device failures
            # surface inside this retry loop, not later in postprocess
            return [{k: np.asarray(v) for k, v in r.items()}
                    for r in res.results]
        except Exception as e:  # rare transient device-unrecoverable flakes
            last_err = e
    raise last_err


def postprocess(results):
    """Merge per-core device outputs into the reference's 4 output arrays."""
    n = N_KPTS
    n_chunks = n // 128
    m_chunks = M_SHARD // 128

    # ---- forward: merge per-core top-2 into global top-2 ----
    m1 = np.empty((N_CORES, n), np.float32)
    m2 = np.empty((N_CORES, n), np.float32)
    for c in range(N_CORES):
        vals = _decode_top8(results[c]["fwd_val"], n_chunks)
        m1[c] = vals[:, 0]
        m2[c] = vals[:, 1]

    w = np.argmax(m1, axis=0)                      # first max on ties
    rows = np.arange(n)
    s1 = m1[w, rows]
    m1_masked = m1.copy()
    m1_masked[w, rows] = -np.inf
    s2 = np.maximum(m1_masked.max(axis=0), m2[w, rows]).astype(np.float32)

    # ---- backward: per-core column max + cross-lane runner-up ----
    cm1 = np.empty(M_KPTS, np.float32)
    cm2 = np.empty(M_KPTS, np.float32)
    lane = np.empty(M_KPTS, np.int64)
    for c in range(N_CORES):
        vals = _decode_top8(results[c]["bwd_val"], m_chunks)
        lanes = _decode_top8(results[c]["bwd_lane"], m_chunks)
        sl = slice(c * M_SHARD, (c + 1) * M_SHARD)
        cm1[sl] = vals[:, 0]
        cm2[sl] = vals[:, 1]
        lane[sl] = lanes[:, 0].astype(np.int64)

    # ---- exact reference arithmetic (float32) ----
    def dist(s):
        return SQRT_2 * np.sqrt(np.maximum(ONE - s, CLIP_LO))

    fd1, fd2 = dist(s1), dist(s2)
    fwd_ok = (fd1 / fd2) < np.float32(1.0)
    bd1, bd2 = dist(cm1), dist(cm2)
    bck_ok = (bd1 / bd2) < np.float32(1.0)

    # ---- mutual NN + index recovery by exact value matching ----
    # Row i matches column j iff s1[i] == cm1[j] (bitwise: both sides are
    # the same f32 S element).  Search row i's winning core's 1024 column
    # maxes for the value s1[i].  A hit from a DIFFERENT dot product that
    # happens to collide bitwise is filtered by the argmax-lane check
    # (i % 128 must equal the column's winning lane); residual ambiguity
    # is reported (expected never).
    fwd_nn = np.zeros(n, np.int64)
    found = np.zeros(n, bool)
    ambig = 0
    for c in range(N_CORES):
        rows_c = np.nonzero(w == c)[0]
        if rows_c.size == 0:
            continue
        M1c = cm1[c * M_SHARD:(c + 1) * M_SHARD]
        order = np.argsort(M1c, kind="stable")
        sv = M1c[order]
        tgt = s1[rows_c]
        lo = np.searchsorted(sv, tgt, side="left")
        hi = np.searchsorted(sv, tgt, side="right")
        lane_c = lane[c * M_SHARD:(c + 1) * M_SHARD]
        for ri, l, h in zip(rows_c, lo, hi):
            if h == l:
                continue
            cands = [j for j in order[l:h] if lane_c[j] == ri % 128]
            if len(cands) == 1:
                fwd_nn[ri] = cands[0] + c * M_SHARD
                found[ri] = True
            elif len(cands) > 1:
                ambig += 1
    if ambig:
        print(f"WARNING: {ambig} ambiguous column-max value collisions")

    mutual = fwd_ok & found & bck_ok[fwd_nn]

    indices0 = np.where(mutual, fwd_nn, -1)[None, :].astype(np.int32)
    mscores0 = (indices0 > 0).astype(np.int32)
    matches1 = np.full((1, M_KPTS), -1, dtype=np.int32)
    mscores1 = np.zeros((1, M_KPTS), dtype=np.float32)
    return indices0, matches1, mscores0, mscores1


def kernel(descriptors0, descriptors1, keypoints0, keypoints1):
    results = run_device(descriptors0, descriptors1)
    return postprocess(results)


# revision 16
# speedup vs baseline: 1.3055x; 1.0178x over previous
"""Trainium2 Bass kernel for Disk descriptor mutual-NN matching (retrieval_knn).

Strategy (8 NeuronCores, shard descriptors1 columns M across cores):
  - Each core c holds full d0 [256, 8192] and its d1 shard [256, 1024].
  - S_c = d0.T @ d1_c via f16x3 split matmuls (3 f16 passes = 3/4 the fp32
    PE time; products exact below fp32 accumulation noise).
  - Forward: per-row top-8 values over the core's 1024 columns (DVE
    InstMax).  No index pass: indices are recovered on the host (below).
  - Backward: per-lane running elementwise max over the 64 row-chunks
    (DVE tensor_max fold, 1 op/chunk) -> fold[p, j] = max over rows
    {p + 128k} of column j.  The [128, 1024] fold state is DMAed out and
    the host reduces over the 128 lanes: exact column max M1[j]
    (bit-identical to the forward S values), its lane, and a cross-lane
    runner-up M2x[j] (== true second unless the column's top-2 share a
    lane, which only matters on exact value ties -- where the reference
    fails the ratio test anyway; with rt=1.0 the ratio test is a pure
    tie detector).
  - Host mutual test: row i is matched to column j iff its top value
    s1[i] IS the column max: s1[i] == M1[j], exact because both sides
    are the same f32 bits from the same PSUM drain.  This also recovers
    the matched column INDEX by value search (unique absent f32 bit
    collisions between distinct dot products, which are detected).
"""

import sys

if "/opt/trn_rl_repo" not in sys.path:
    sys.path.insert(0, "/opt/trn_rl_repo")

import numpy as np

N_KPTS = 8192
M_KPTS = 8192
F_DIM = 256
N_CORES = 8
M_SHARD = M_KPTS // N_CORES  # 1024

SQRT_2 = np.float32(1.414213)
CLIP_LO = np.float32(1e-6)
ONE = np.float32(1.0)

DTYPE_MODE = "f16x3"


def _split_f16(a32):
    """f32 -> (h, h/32, 32*(a-h)) as float16, with f16-subnormal highs
    flushed into the residual so no information rides on f16 subnormals."""
    h = a32.astype(np.float16)
    h[np.abs(a32) < 6.104e-5] = np.float16(0)
    l = a32 - h.astype(np.float32)
    h_s = (h.astype(np.float32) / 32.0).astype(np.float16)
    l_s = (l * 32.0).astype(np.float16)
    return h, h_s, l_s


# --------------------------------------------------------------------------
# Device kernel builder
# --------------------------------------------------------------------------

def build_kernel(n_rows=N_KPTS, m_shard=M_SHARD, f_dim=F_DIM,
                 dtype_mode=DTYPE_MODE):
    """Build the per-core SPMD Bass program.

    Inputs (per core, f16 split terms):
      d0h/d0hs/d0ls: [kf, 128, n_rows]   (descriptors0, K-chunked)
      d1h/d1hs/d1ls: [kf, 128, m_shard]  (this core's descriptors1 shard)
    Outputs (per core):
      fwd_val [128, n_chunks*8] f32  (row top-8 per 128-row chunk)
      fold    [128, m_shard] f32     (per-lane column maxes)
    """
    import concourse.bacc as bacc
    import concourse.mybir as mybir
    import concourse.tile as tile
    from concourse.masks import make_identity

    kf = f_dim // 128
    n_chunks = n_rows // 128          # row chunks
    m_tiles = max(1, m_shard // 512)  # 512-wide column tiles (PSUM banks)
    mw = min(512, m_shard)
    m_chunks = m_shard // 128         # backward column chunks
    assert m_shard % 128 == 0 and f_dim % 128 == 0

    nc = bacc.Bacc("TRN2", target_bir_lowering=False, debug=False,
                   num_devices=1)

    if dtype_mode == "fp32":
        in_names = ["d0", "d1"]
        in_dt = mybir.dt.float32
    else:
        in_names = ["d0h", "d0hs", "d0ls", "d1h", "d1hs", "d1ls"]
        in_dt = mybir.dt.float16
    in_dram = {}
    for nm in in_names:
        nw = n_rows if nm.startswith("d0") else m_shard
        in_dram[nm] = nc.dram_tensor(nm, [kf, 128, nw], in_dt,
                                     kind="ExternalInput")
    fwd_val = nc.dram_tensor("fwd_val", [128, n_chunks * 8], mybir.dt.float32,
                             kind="ExternalOutput")
    fold_out = nc.dram_tensor("fold", [128, m_shard], mybir.dt.float32,
                              kind="ExternalOutput")

    with tile.TileContext(nc) as tc:
        with tc.tile_pool(name="persist", bufs=1) as persist, \
             tc.tile_pool(name="schunk", bufs=4) as schunk_pool, \
             tc.tile_pool(name="outs", bufs=1) as outs_pool, \
             tc.tile_pool(name="psf", bufs=2, space="PSUM") as psf:

            # resident inputs; d0 loads split along n so early fwd units
            # unblock before the full load completes
            in_sb = {}
            for nm in in_names:
                nw = n_rows if nm.startswith("d0") else m_shard
                in_sb[nm] = [persist.tile([128, nw], in_dt,
                                          name=f"{nm}sb{k}", tag=f"{nm}sb{k}")
                             for k in range(kf)]
            # load order: exactly the tiles the first row-chunk's matmuls
            # consume, smallest first, then the bulk d0 stream
            n_split = 8 if n_rows % 1024 == 0 else 1
            d0n = [nm for nm in in_names if nm.startswith("d0")]
            d1n = [nm for nm in in_names if nm.startswith("d1")]

            def dma(nm, k, sl):
                nc.sync.dma_start(in_sb[nm][k][:, sl], in_dram[nm][k][:, sl])

            half = slice(0, mw)
            rest = slice(mw, m_shard) if m_shard > mw else None
            tiny = slice(0, 128)
            for k in range(kf):
                if dtype_mode == "fp32":
                    dma("d1", k, half)
                    dma("d0", k, tiny)
                    if rest:
                        dma("d1", k, rest)
                else:
                    dma("d1h", k, half)
                    dma("d0h", k, tiny)
                    if rest:
                        dma("d1h", k, rest)
                    dma("d1ls", k, slice(0, m_shard))
                    dma("d0hs", k, tiny)
                    dma("d1hs", k, slice(0, m_shard))
                    dma("d0ls", k, tiny)
            for p in range(n_split):
                sl = slice(max(128, p * n_rows // n_split),
                           (p + 1) * n_rows // n_split) if p == 0 else \
                     slice(p * n_rows // n_split, (p + 1) * n_rows // n_split)
                for nm in d0n:
                    for k in range(kf):
                        dma(nm, k, sl)
            if dtype_mode == "fp32":
                terms = [("d0", "d1")]
            else:
                terms = [("d0h", "d1h"), ("d0hs", "d1ls"), ("d0ls", "d1hs")]

            ident = persist.tile([128, 128], mybir.dt.float32, name="ident")
            make_identity(nc, ident[:])
            # warm-up matmul: starts the PE p-state ramp clock while input
            # DMAs are still streaming (identity needs no DMA)
            warm = psf.tile([128, 8], mybir.dt.float32, tag="pf", name="warm",
                            padded_shape=[128, m_tiles * mw])
            nc.tensor.matmul(warm[:], ident[:], ident[:, :8],
                             start=True, stop=True)

            fv_sb = outs_pool.tile([128, n_chunks * 8], mybir.dt.float32)
            # per-lane running column max over row chunks
            fold = outs_pool.tile([128, m_shard], mybir.dt.float32,
                                  name="fold")

            n_acc = kf * len(terms)
            for n in range(n_chunks):
                last = n == n_chunks - 1
                # one PSUM tile spanning m_tiles banks; each matmul
                # writes within a single bank; one wide ACT copy drains
                pf = psf.tile([128, m_tiles * mw], mybir.dt.float32,
                              tag="pf", name="pf")
                for k in range(kf):
                    for ti, (lnm, rnm) in enumerate(terms):
                        for m in range(m_tiles):
                            acc = k * len(terms) + ti
                            nc.tensor.matmul(
                                pf[:, m * mw:(m + 1) * mw],
                                in_sb[lnm][k][:, n * 128:(n + 1) * 128],
                                in_sb[rnm][k][:, m * mw:(m + 1) * mw],
                                start=(acc == 0), stop=(acc == n_acc - 1))
                if last:
                    # final chunk reads PSUM directly: its fold gates the
                    # fold DMA, so skip the ACT copy latency
                    src = pf
                else:
                    src = schunk_pool.tile([128, m_shard],
                                           mybir.dt.float32, tag="schunk")
                    nc.scalar.copy(src[:], pf[:])
                # backward fold first: it is the tail's dependence chain
                if n == 0:
                    nc.vector.tensor_copy(fold[:], src[:])
                else:
                    nc.vector.tensor_max(fold[:], fold[:], src[:])
                if last:
                    for q in range(4):
                        sl = slice(q * m_shard // 4, (q + 1) * m_shard // 4)
                        nc.sync.dma_start(fold_out[:, sl], fold[:, sl])
                nc.vector.max(out=fv_sb[:, n * 8:(n + 1) * 8],
                              in_=src[:])
                # stream forward outputs out as they complete
                if (n + 1) % (n_chunks // 4) == 0:
                    sl = slice((n + 1 - n_chunks // 4) * 8, (n + 1) * 8)
                    nc.sync.dma_start(fwd_val[:, sl], fv_sb[:, sl])

    nc.compile()
    return nc


_KERNEL_CACHE = {}


def get_kernel(dtype_mode=DTYPE_MODE):
    if dtype_mode not in _KERNEL_CACHE:
        _KERNEL_CACHE[dtype_mode] = build_kernel(dtype_mode=dtype_mode)
    return _KERNEL_CACHE[dtype_mode]


# --------------------------------------------------------------------------
# Host side
# --------------------------------------------------------------------------

def _decode_top8(arr, chunks):
    """[128, chunks*8] -> [chunks*128, 8] with row r = chunk*128 + partition."""
    return arr.reshape(128, chunks, 8).transpose(1, 0, 2).reshape(chunks * 128, 8)


def run_device(descriptors0, descriptors1, dtype_mode=DTYPE_MODE):
    """Run the SPMD kernel on 8 cores. Returns per-core raw outputs."""
    from concourse.bass_utils import run_bass_kernel_spmd

    nc = get_kernel(dtype_mode)
    d0 = np.ascontiguousarray(descriptors0[0]).astype(np.float32, copy=False)
    d1 = np.ascontiguousarray(descriptors1[0]).astype(np.float32, copy=False)
    kf = F_DIM // 128

    def shard(a, c):
        return np.ascontiguousarray(
            a[:, c * M_SHARD:(c + 1) * M_SHARD]).reshape(kf, 128, M_SHARD)

    if dtype_mode == "fp32":
        d0r = d0.reshape(kf, 128, N_KPTS)
        in_maps = [{"d0": d0r, "d1": shard(d1, c)} for c in range(N_CORES)]
    else:
        h0, h0s, l0s = _split_f16(d0)
        h1, h1s, l1s = _split_f16(d1)
        d0m = {"d0h": h0.reshape(kf, 128, N_KPTS),
               "d0hs": h0s.reshape(kf, 128, N_KPTS),
               "d0ls": l0s.reshape(kf, 128, N_KPTS)}
        in_maps = [dict(d0m, d1h=shard(h1, c), d1hs=shard(h1s, c),
                        d1ls=shard(l1s, c)) for c in range(N_CORES)]
    last_err = None
    for _attempt in range(3):
        try:
            res = run_bass_kernel_spmd(nc, in_maps, list(range(N_CORES)))
            # materialize device arrays now so transient device failures
            # surface inside this retry loop, not later in postprocess
            return [{k: np.asarray(v) for k, v in r.items()}
                    for r in res.results]
        except Exception as e:  # rare transient device-unrecoverable flakes
            last_err = e
    raise last_err


def postprocess(results):
    """Merge per-core device outputs into the reference's 4 output arrays."""
    n = N_KPTS
    n_chunks = n // 128
    m_chunks = M_SHARD // 128

    # ---- forward: merge per-core top-2 into global top-2 ----
    m1 = np.empty((N_CORES, n), np.float32)
    m2 = np.empty((N_CORES, n), np.float32)
    for c in range(N_CORES):
        vals = _decode_top8(results[c]["fwd_val"], n_chunks)
        m1[c] = vals[:, 0]
        m2[c] = vals[:, 1]

    w = np.argmax(m1, axis=0)                      # first max on ties
    rows = np.arange(n)
    s1 = m1[w, rows]
    m1_masked = m1.copy()
    m1_masked[w, rows] = -np.inf
    s2 = np.maximum(m1_masked.max(axis=0), m2[w, rows]).astype(np.float32)

    # ---- backward: reduce the per-lane fold over its 128 lanes ----
    cm1 = np.empty(M_KPTS, np.float32)
    cm2 = np.empty(M_KPTS, np.float32)
    lane = np.empty(M_KPTS, np.int64)
    for c in range(N_CORES):
        f = results[c]["fold"]                     # [128, M_SHARD]
        sl = slice(c * M_SHARD, (c + 1) * M_SHARD)
        lane[sl] = np.argmax(f, axis=0)
        cm1[sl] = f[lane[sl], np.arange(M_SHARD)]
        cm2[sl] = np.partition(f, 126, axis=0)[126]

    # ---- exact reference arithmetic (float32) ----
    def dist(s):
        return SQRT_2 * np.sqrt(np.maximum(ONE - s, CLIP_LO))

    fd1, fd2 = dist(s1), dist(s2)
    fwd_ok = (fd1 / fd2) < np.float32(1.0)
    bd1, bd2 = dist(cm1), dist(cm2)
    bck_ok = (bd1 / bd2) < np.float32(1.0)

    # ---- mutual NN + index recovery by exact value matching ----
    # Row i matches column j iff s1[i] == cm1[j] (bitwise: both sides are
    # the same f32 S element).  Search row i's winning core's 1024 column
    # maxes for the value s1[i].  A hit from a DIFFERENT dot product that
    # happens to collide bitwise is filtered by the argmax-lane check
    # (i % 128 must equal the column's winning lane); residual ambiguity
    # is reported (expected never).
    fwd_nn = np.zeros(n, np.int64)
    found = np.zeros(n, bool)
    ambig = 0
    for c in range(N_CORES):
        rows_c = np.nonzero(w == c)[0]
        if rows_c.size == 0:
            continue
        M1c = cm1[c * M_SHARD:(c + 1) * M_SHARD]
        order = np.argsort(M1c, kind="stable")
        sv = M1c[order]
        tgt = s1[rows_c]
        lo = np.searchsorted(sv, tgt, side="left")
        hi = np.searchsorted(sv, tgt, side="right")
        lane_c = lane[c * M_SHARD:(c + 1) * M_SHARD]
        for ri, l, h in zip(rows_c, lo, hi):
            if h == l:
                continue
            cands = [j for j in order[l:h] if lane_c[j] == ri % 128]
            if len(cands) == 1:
                fwd_nn[ri] = cands[0] + c * M_SHARD
                found[ri] = True
            elif len(cands) > 1:
                ambig += 1
    if ambig:
        print(f"WARNING: {ambig} ambiguous column-max value collisions")

    mutual = fwd_ok & found & bck_ok[fwd_nn]

    indices0 = np.where(mutual, fwd_nn, -1)[None, :].astype(np.int32)
    mscores0 = (indices0 > 0).astype(np.int32)
    matches1 = np.full((1, M_KPTS), -1, dtype=np.int32)
    mscores1 = np.zeros((1, M_KPTS), dtype=np.float32)
    return indices0, matches1, mscores0, mscores1


def kernel(descriptors0, descriptors1, keypoints0, keypoints1):
    results = run_device(descriptors0, descriptors1)
    return postprocess(results)


# revision 21
# speedup vs baseline: 1.3544x; 1.0375x over previous
"""Trainium2 Bass kernel for Disk descriptor mutual-NN matching (retrieval_knn).

Strategy (8 NeuronCores, shard descriptors1 columns M across cores):
  - Each core c holds full d0 [256, 8192] and its d1 shard [256, 1024].
  - S_c = d0.T @ d1_c via f16x3 split matmuls (3 f16 passes = 3/4 the fp32
    PE time; products exact below fp32 accumulation noise).
  - Forward: per-row top-8 values over the core's 1024 columns (DVE
    InstMax).  No index pass: indices are recovered on the host (below).
  - Backward: per-lane running elementwise max over the 64 row-chunks
    (DVE tensor_max fold, 1 op/chunk) -> fold[p, j] = max over rows
    {p + 128k} of column j.  The [128, 1024] fold state is DMAed out and
    the host reduces over the 128 lanes: exact column max M1[j]
    (bit-identical to the forward S values), its lane, and a cross-lane
    runner-up M2x[j] (== true second unless the column's top-2 share a
    lane, which only matters on exact value ties -- where the reference
    fails the ratio test anyway; with rt=1.0 the ratio test is a pure
    tie detector).
  - Host mutual test: row i is matched to column j iff its top value
    s1[i] IS the column max: s1[i] == M1[j], exact because both sides
    are the same f32 bits from the same PSUM drain.  This also recovers
    the matched column INDEX by value search (unique absent f32 bit
    collisions between distinct dot products, which are detected).
"""

import sys

if "/opt/trn_rl_repo" not in sys.path:
    sys.path.insert(0, "/opt/trn_rl_repo")

import numpy as np

N_KPTS = 8192
M_KPTS = 8192
F_DIM = 256
N_CORES = 8
M_SHARD = M_KPTS // N_CORES  # 1024

SQRT_2 = np.float32(1.414213)
CLIP_LO = np.float32(1e-6)
ONE = np.float32(1.0)

DTYPE_MODE = "f16x3"


def _split_f16(a32):
    """f32 -> (h, h/32, 32*(a-h)) as float16, with f16-subnormal highs
    flushed into the residual so no information rides on f16 subnormals."""
    h = a32.astype(np.float16)
    h[np.abs(a32) < 6.104e-5] = np.float16(0)
    l = a32 - h.astype(np.float32)
    h_s = (h.astype(np.float32) / 32.0).astype(np.float16)
    l_s = (l * 32.0).astype(np.float16)
    return h, h_s, l_s


# --------------------------------------------------------------------------
# Device kernel builder
# --------------------------------------------------------------------------

def build_kernel(n_rows=N_KPTS, m_shard=M_SHARD, f_dim=F_DIM,
                 dtype_mode=DTYPE_MODE):
    """Build the per-core SPMD Bass program.

    Inputs (per core, f16 split terms):
      d0h/d0hs/d0ls: [kf, 128, n_rows]   (descriptors0, K-chunked)
      d1h/d1hs/d1ls: [kf, 128, m_shard]  (this core's descriptors1 shard)
    Outputs (per core):
      fwd_val [128, n_chunks*8] f32  (row top-8 per 128-row chunk)
      fold    [128, m_shard] f32     (per-lane column maxes)
    """
    import concourse.bacc as bacc
    import concourse.mybir as mybir
    import concourse.tile as tile

    kf = f_dim // 128
    n_chunks = n_rows // 128          # row chunks
    m_tiles = max(1, m_shard // 512)  # 512-wide column tiles (PSUM banks)
    mw = min(512, m_shard)
    m_chunks = m_shard // 128         # backward column chunks
    assert m_shard % 128 == 0 and f_dim % 128 == 0

    nc = bacc.Bacc("TRN2", target_bir_lowering=False, debug=False,
                   num_devices=1)

    if dtype_mode == "fp32":
        in_names = ["d0", "d1"]
        in_dt = mybir.dt.float32
    else:
        in_names = ["d0h", "d0hs", "d0ls", "d1h", "d1hs", "d1ls"]
        in_dt = mybir.dt.float16
    in_dram = {}
    for nm in in_names:
        nw = n_rows if nm.startswith("d0") else m_shard
        in_dram[nm] = nc.dram_tensor(nm, [kf, 128, nw], in_dt,
                                     kind="ExternalInput")
    fwd_val = nc.dram_tensor("fwd_val", [128, n_chunks * 8], mybir.dt.float32,
                             kind="ExternalOutput")
    fold_out = nc.dram_tensor("fold", [128, m_shard], mybir.dt.float32,
                              kind="ExternalOutput")

    with tile.TileContext(nc) as tc:
        with tc.tile_pool(name="persist", bufs=1) as persist, \
             tc.tile_pool(name="schunk", bufs=4) as schunk_pool, \
             tc.tile_pool(name="outs", bufs=1) as outs_pool, \
             tc.tile_pool(name="psf", bufs=2, space="PSUM") as psf:

            # resident inputs; d0 loads split along n so early fwd units
            # unblock before the full load completes
            in_sb = {}
            for nm in in_names:
                nw = n_rows if nm.startswith("d0") else m_shard
                in_sb[nm] = [persist.tile([128, nw], in_dt,
                                          name=f"{nm}sb{k}", tag=f"{nm}sb{k}")
                             for k in range(kf)]
            # input loads: d1 shards issue from the otherwise-idle ACT /
            # DVE / Pool sequencers in parallel with the d0 piece stream
            # on SP, so the 12 tiles chunk 0 consumes are all in flight
            # within ~1 us instead of serializing on SP's 565 ns issue
            n_split = 8 if n_rows % 1024 == 0 else 1
            d0n = [nm for nm in in_names if nm.startswith("d0")]
            d1n = [nm for nm in in_names if nm.startswith("d1")]

            d1_eng = {"d1h": nc.scalar, "d1hs": nc.scalar, "d1ls": nc.gpsimd,
                      "d1": nc.scalar}
            for k in range(kf):
                for nm in d1n:
                    d1_eng[nm].dma_start(in_sb[nm][k][:], in_dram[nm][k])
            for p in range(n_split):
                sl = slice(p * n_rows // n_split, (p + 1) * n_rows // n_split)
                for k in range(kf):
                    for nm in d0n:
                        nc.sync.dma_start(in_sb[nm][k][:, sl],
                                          in_dram[nm][k][:, sl])
            if dtype_mode == "fp32":
                terms = [("d0", "d1")]
            else:
                terms = [("d0h", "d1h"), ("d0hs", "d1ls"), ("d0ls", "d1hs")]

            zeros = persist.tile([128, 128], mybir.dt.float32, name="zeros")
            nc.vector.memset(zeros[:], 0)
            # warm-up matmul: starts the PE p-state ramp clock while input
            # DMAs are still streaming (zeros need no DMA)
            warm = psf.tile([128, 8], mybir.dt.float32, tag="pf", name="warm",
                            padded_shape=[128, m_tiles * mw])
            nc.tensor.matmul(warm[:], zeros[:], zeros[:, :8],
                             start=True, stop=True)

            fv_sb = outs_pool.tile([128, n_chunks * 8], mybir.dt.float32)
            # per-lane running column max over row chunks
            fold = outs_pool.tile([128, m_shard], mybir.dt.float32,
                                  name="fold")

            n_acc = kf * len(terms)
            for n in range(n_chunks):
                last = n == n_chunks - 1
                # one PSUM tile spanning m_tiles banks; each matmul
                # writes within a single bank; one wide ACT copy drains
                pf = psf.tile([128, m_tiles * mw], mybir.dt.float32,
                              tag="pf", name="pf")
                for k in range(kf):
                    for ti, (lnm, rnm) in enumerate(terms):
                        for m in range(m_tiles):
                            acc = k * len(terms) + ti
                            nc.tensor.matmul(
                                pf[:, m * mw:(m + 1) * mw],
                                in_sb[lnm][k][:, n * 128:(n + 1) * 128],
                                in_sb[rnm][k][:, m * mw:(m + 1) * mw],
                                start=(acc == 0), stop=(acc == n_acc - 1))
                if last:
                    # final chunk reads PSUM directly: its fold gates the
                    # fold DMA, so skip the ACT copy latency
                    src = pf
                else:
                    src = schunk_pool.tile([128, m_shard],
                                           mybir.dt.float32, tag="schunk")
                    nc.scalar.copy(src[:], pf[:])
                # backward fold first: it is the tail's dependence chain
                if n == 0:
                    nc.vector.tensor_copy(fold[:], src[:])
                else:
                    nc.vector.tensor_max(fold[:], fold[:], src[:])
                if last:
                    # fold DMA goes out before the deferred fv maxes
                    for q in range(2):
                        sl = slice(q * m_shard // 2, (q + 1) * m_shard // 2)
                        nc.sync.dma_start(fold_out[:, sl], fold[:, sl])
                    for pn, psrc in deferred:
                        nc.vector.max(out=fv_sb[:, pn * 8:(pn + 1) * 8],
                                      in_=psrc[:])
                    nc.vector.max(out=fv_sb[:, n * 8:(n + 1) * 8],
                                  in_=src[:])
                elif n == n_chunks - 2:
                    # defer this max until after the final fold so the
                    # fold chain is not delayed by an unrelated DVE op
                    deferred = [(n, src)]
                else:
                    nc.vector.max(out=fv_sb[:, n * 8:(n + 1) * 8],
                                  in_=src[:])
                # stream forward outputs out as they complete
                if (n + 1) % (n_chunks // 4) == 0 and n != n_chunks - 1:
                    sl = slice((n + 1 - n_chunks // 4) * 8, (n + 1) * 8)
                    nc.sync.dma_start(fwd_val[:, sl], fv_sb[:, sl])
            sl = slice((n_chunks - n_chunks // 4) * 8, n_chunks * 8)
            nc.sync.dma_start(fwd_val[:, sl], fv_sb[:, sl])

    nc.compile()
    return nc


_KERNEL_CACHE = {}


def get_kernel(dtype_mode=DTYPE_MODE):
    if dtype_mode not in _KERNEL_CACHE:
        _KERNEL_CACHE[dtype_mode] = build_kernel(dtype_mode=dtype_mode)
    return _KERNEL_CACHE[dtype_mode]


# --------------------------------------------------------------------------
# Host side
# --------------------------------------------------------------------------

def _decode_top8(arr, chunks):
    """[128, chunks*8] -> [chunks*128, 8] with row r = chunk*128 + partition."""
    return arr.reshape(128, chunks, 8).transpose(1, 0, 2).reshape(chunks * 128, 8)


def run_device(descriptors0, descriptors1, dtype_mode=DTYPE_MODE):
    """Run the SPMD kernel on 8 cores. Returns per-core raw outputs."""
    from concourse.bass_utils import run_bass_kernel_spmd

    nc = get_kernel(dtype_mode)
    d0 = np.ascontiguousarray(descriptors0[0]).astype(np.float32, copy=False)
    d1 = np.ascontiguousarray(descriptors1[0]).astype(np.float32, copy=False)
    kf = F_DIM // 128

    def shard(a, c):
        return np.ascontiguousarray(
            a[:, c * M_SHARD:(c + 1) * M_SHARD]).reshape(kf, 128, M_SHARD)

    if dtype_mode == "fp32":
        d0r = d0.reshape(kf, 128, N_KPTS)
        in_maps = [{"d0": d0r, "d1": shard(d1, c)} for c in range(N_CORES)]
    else:
        h0, h0s, l0s = _split_f16(d0)
        h1, h1s, l1s = _split_f16(d1)
        d0m = {"d0h": h0.reshape(kf, 128, N_KPTS),
               "d0hs": h0s.reshape(kf, 128, N_KPTS),
               "d0ls": l0s.reshape(kf, 128, N_KPTS)}
        in_maps = [dict(d0m, d1h=shard(h1, c), d1hs=shard(h1s, c),
                        d1ls=shard(l1s, c)) for c in range(N_CORES)]
    last_err = None
    for _attempt in range(3):
        try:
            res = run_bass_kernel_spmd(nc, in_maps, list(range(N_CORES)))
            # materialize device arrays now so transient device failures
            # surface inside this retry loop, not later in postprocess
            return [{k: np.asarray(v) for k, v in r.items()}
                    for r in res.results]
        except Exception as e:  # rare transient device-unrecoverable flakes
            last_err = e
    raise last_err


def postprocess(results):
    """Merge per-core device outputs into the reference's 4 output arrays."""
    n = N_KPTS
    n_chunks = n // 128
    m_chunks = M_SHARD // 128

    # ---- forward: merge per-core top-2 into global top-2 ----
    m1 = np.empty((N_CORES, n), np.float32)
    m2 = np.empty((N_CORES, n), np.float32)
    for c in range(N_CORES):
        vals = _decode_top8(results[c]["fwd_val"], n_chunks)
        m1[c] = vals[:, 0]
        m2[c] = vals[:, 1]

    w = np.argmax(m1, axis=0)                      # first max on ties
    rows = np.arange(n)
    s1 = m1[w, rows]
    m1_masked = m1.copy()
    m1_masked[w, rows] = -np.inf
    s2 = np.maximum(m1_masked.max(axis=0), m2[w, rows]).astype(np.float32)

    # ---- backward: reduce the per-lane fold over its 128 lanes ----
    cm1 = np.empty(M_KPTS, np.float32)
    cm2 = np.empty(M_KPTS, np.float32)
    lane = np.empty(M_KPTS, np.int64)
    for c in range(N_CORES):
        f = results[c]["fold"]                     # [128, M_SHARD]
        sl = slice(c * M_SHARD, (c + 1) * M_SHARD)
        lane[sl] = np.argmax(f, axis=0)
        cm1[sl] = f[lane[sl], np.arange(M_SHARD)]
        cm2[sl] = np.partition(f, 126, axis=0)[126]

    # ---- exact reference arithmetic (float32) ----
    def dist(s):
        return SQRT_2 * np.sqrt(np.maximum(ONE - s, CLIP_LO))

    fd1, fd2 = dist(s1), dist(s2)
    fwd_ok = (fd1 / fd2) < np.float32(1.0)
    bd1, bd2 = dist(cm1), dist(cm2)
    bck_ok = (bd1 / bd2) < np.float32(1.0)

    # ---- mutual NN + index recovery by exact value matching ----
    # Row i matches column j iff s1[i] == cm1[j] (bitwise: both sides are
    # the same f32 S element).  Search row i's winning core's 1024 column
    # maxes for the value s1[i].  A hit from a DIFFERENT dot product that
    # happens to collide bitwise is filtered by the argmax-lane check
    # (i % 128 must equal the column's winning lane); residual ambiguity
    # is reported (expected never).
    fwd_nn = np.zeros(n, np.int64)
    found = np.zeros(n, bool)
    ambig = 0
    for c in range(N_CORES):
        rows_c = np.nonzero(w == c)[0]
        if rows_c.size == 0:
            continue
        M1c = cm1[c * M_SHARD:(c + 1) * M_SHARD]
        order = np.argsort(M1c, kind="stable")
        sv = M1c[order]
        tgt = s1[rows_c]
        lo = np.searchsorted(sv, tgt, side="left")
        hi = np.searchsorted(sv, tgt, side="right")
        lane_c = lane[c * M_SHARD:(c + 1) * M_SHARD]
        for ri, l, h in zip(rows_c, lo, hi):
            if h == l:
                continue
            cands = [j for j in order[l:h] if lane_c[j] == ri % 128]
            if len(cands) == 1:
                fwd_nn[ri] = cands[0] + c * M_SHARD
                found[ri] = True
            elif len(cands) > 1:
                ambig += 1
    if ambig:
        print(f"WARNING: {ambig} ambiguous column-max value collisions")

    mutual = fwd_ok & found & bck_ok[fwd_nn]

    indices0 = np.where(mutual, fwd_nn, -1)[None, :].astype(np.int32)
    mscores0 = (indices0 > 0).astype(np.int32)
    matches1 = np.full((1, M_KPTS), -1, dtype=np.int32)
    mscores1 = np.zeros((1, M_KPTS), dtype=np.float32)
    return indices0, matches1, mscores0, mscores1


def kernel(descriptors0, descriptors1, keypoints0, keypoints1):
    results = run_device(descriptors0, descriptors1)
    return postprocess(results)


# revision 23
# speedup vs baseline: 1.3571x; 1.0020x over previous
"""Trainium2 Bass kernel for Disk descriptor mutual-NN matching (retrieval_knn).

Strategy (8 NeuronCores, shard descriptors1 columns M across cores):
  - Each core c holds full d0 [256, 8192] and its d1 shard [256, 1024].
  - S_c = d0.T @ d1_c via f16x3 split matmuls (3 f16 passes = 3/4 the fp32
    PE time; products exact below fp32 accumulation noise).
  - Forward: per-row top-8 values over the core's 1024 columns (DVE
    InstMax).  No index pass: indices are recovered on the host (below).
  - Backward: per-lane running elementwise max over the 64 row-chunks
    (DVE tensor_max fold, 1 op/chunk) -> fold[p, j] = max over rows
    {p + 128k} of column j.  The [128, 1024] fold state is DMAed out and
    the host reduces over the 128 lanes: exact column max M1[j]
    (bit-identical to the forward S values), its lane, and a cross-lane
    runner-up M2x[j] (== true second unless the column's top-2 share a
    lane, which only matters on exact value ties -- where the reference
    fails the ratio test anyway; with rt=1.0 the ratio test is a pure
    tie detector).
  - Host mutual test: row i is matched to column j iff its top value
    s1[i] IS the column max: s1[i] == M1[j], exact because both sides
    are the same f32 bits from the same PSUM drain.  This also recovers
    the matched column INDEX by value search (unique absent f32 bit
    collisions between distinct dot products, which are detected).
"""

import sys

if "/opt/trn_rl_repo" not in sys.path:
    sys.path.insert(0, "/opt/trn_rl_repo")

import numpy as np

N_KPTS = 8192
M_KPTS = 8192
F_DIM = 256
N_CORES = 8
M_SHARD = M_KPTS // N_CORES  # 1024

SQRT_2 = np.float32(1.414213)
CLIP_LO = np.float32(1e-6)
ONE = np.float32(1.0)

DTYPE_MODE = "f16x3"


def _split_f16(a32):
    """f32 -> (h, h/32, 32*(a-h)) as float16, with f16-subnormal highs
    flushed into the residual so no information rides on f16 subnormals."""
    h = a32.astype(np.float16)
    h[np.abs(a32) < 6.104e-5] = np.float16(0)
    l = a32 - h.astype(np.float32)
    h_s = (h.astype(np.float32) / 32.0).astype(np.float16)
    l_s = (l * 32.0).astype(np.float16)
    return h, h_s, l_s


# --------------------------------------------------------------------------
# Device kernel builder
# --------------------------------------------------------------------------

def build_kernel(n_rows=N_KPTS, m_shard=M_SHARD, f_dim=F_DIM,
                 dtype_mode=DTYPE_MODE):
    """Build the per-core SPMD Bass program.

    Inputs (per core, f16 split terms):
      d0h/d0hs/d0ls: [kf, 128, n_rows]   (descriptors0, K-chunked)
      d1h/d1hs/d1ls: [kf, 128, m_shard]  (this core's descriptors1 shard)
    Outputs (per core):
      fwd_val [128, n_chunks*8] f32  (row top-8 per 128-row chunk)
      fold    [128, m_shard] f32     (per-lane column maxes)
    """
    import concourse.bacc as bacc
    import concourse.mybir as mybir
    import concourse.tile as tile

    kf = f_dim // 128
    n_chunks = n_rows // 128          # row chunks
    m_tiles = max(1, m_shard // 512)  # 512-wide column tiles (PSUM banks)
    mw = min(512, m_shard)
    m_chunks = m_shard // 128         # backward column chunks
    assert m_shard % 128 == 0 and f_dim % 128 == 0

    nc = bacc.Bacc("TRN2", target_bir_lowering=False, debug=False,
                   num_devices=1)

    if dtype_mode == "fp32":
        in_names = ["d0", "d1"]
        in_dt = mybir.dt.float32
    else:
        in_names = ["d0h", "d0hs", "d0ls", "d1h", "d1hs", "d1ls"]
        in_dt = mybir.dt.float16
    in_dram = {}
    for nm in in_names:
        nw = n_rows if nm.startswith("d0") else m_shard
        in_dram[nm] = nc.dram_tensor(nm, [kf, 128, nw], in_dt,
                                     kind="ExternalInput")
    fwd_val = nc.dram_tensor("fwd_val", [128, n_chunks * 8], mybir.dt.float32,
                             kind="ExternalOutput")
    fold_out = nc.dram_tensor("fold", [128, m_shard], mybir.dt.float32,
                              kind="ExternalOutput")

    with tile.TileContext(nc) as tc:
        with tc.tile_pool(name="persist", bufs=1) as persist, \
             tc.tile_pool(name="schunk", bufs=4) as schunk_pool, \
             tc.tile_pool(name="outs", bufs=1) as outs_pool, \
             tc.tile_pool(name="psf", bufs=3, space="PSUM") as psf:

            # resident inputs; d0 loads split along n so early fwd units
            # unblock before the full load completes
            in_sb = {}
            for nm in in_names:
                nw = n_rows if nm.startswith("d0") else m_shard
                in_sb[nm] = [persist.tile([128, nw], in_dt,
                                          name=f"{nm}sb{k}", tag=f"{nm}sb{k}")
                             for k in range(kf)]
            # input loads: d1 shards issue from the otherwise-idle ACT /
            # DVE / Pool sequencers in parallel with the d0 piece stream
            # on SP, so the 12 tiles chunk 0 consumes are all in flight
            # within ~1 us instead of serializing on SP's 565 ns issue
            n_split = 8 if n_rows % 1024 == 0 else 1
            d0n = [nm for nm in in_names if nm.startswith("d0")]
            d1n = [nm for nm in in_names if nm.startswith("d1")]

            d1_eng = {"d1h": nc.scalar, "d1hs": nc.scalar, "d1ls": nc.gpsimd,
                      "d1": nc.scalar}
            for k in range(kf):
                for nm in d1n:
                    d1_eng[nm].dma_start(in_sb[nm][k][:], in_dram[nm][k])
            for p in range(n_split):
                sl = slice(p * n_rows // n_split, (p + 1) * n_rows // n_split)
                for k in range(kf):
                    for nm in d0n:
                        nc.sync.dma_start(in_sb[nm][k][:, sl],
                                          in_dram[nm][k][:, sl])
            if dtype_mode == "fp32":
                terms = [("d0", "d1")]
            else:
                terms = [("d0h", "d1h"), ("d0hs", "d1ls"), ("d0ls", "d1hs")]

            zeros = persist.tile([128, 128], mybir.dt.float32, name="zeros")
            nc.vector.memset(zeros[:], 0)
            # warm-up matmul: starts the PE p-state ramp clock while input
            # DMAs are still streaming (zeros need no DMA)
            warm = psf.tile([128, 8], mybir.dt.float32, tag="pf", name="warm",
                            padded_shape=[128, m_tiles * mw])
            nc.tensor.matmul(warm[:], zeros[:], zeros[:, :8],
                             start=True, stop=True)

            fv_sb = outs_pool.tile([128, n_chunks * 8], mybir.dt.float32)
            # per-lane running column max over row chunks
            fold = outs_pool.tile([128, m_shard], mybir.dt.float32,
                                  name="fold")

            n_acc = kf * len(terms)
            for n in range(n_chunks):
                last = n == n_chunks - 1
                # one PSUM tile spanning m_tiles banks; each matmul
                # writes within a single bank; one wide ACT copy drains
                pf = psf.tile([128, m_tiles * mw], mybir.dt.float32,
                              tag="pf", name="pf")
                for k in range(kf):
                    for ti, (lnm, rnm) in enumerate(terms):
                        for m in range(m_tiles):
                            acc = k * len(terms) + ti
                            nc.tensor.matmul(
                                pf[:, m * mw:(m + 1) * mw],
                                in_sb[lnm][k][:, n * 128:(n + 1) * 128],
                                in_sb[rnm][k][:, m * mw:(m + 1) * mw],
                                start=(acc == 0), stop=(acc == n_acc - 1))
                if n >= n_chunks - 2:
                    # final chunks read PSUM directly: their folds gate the
                    # fold DMA, so skip the ACT copy phase lag
                    src = pf
                else:
                    src = schunk_pool.tile([128, m_shard],
                                           mybir.dt.float32, tag="schunk")
                    nc.scalar.copy(src[:], pf[:])
                # backward fold first: it is the tail's dependence chain
                if n == 0:
                    nc.vector.tensor_copy(fold[:], src[:])
                else:
                    nc.vector.tensor_max(fold[:], fold[:], src[:])
                if last:
                    for q in range(2):
                        sl = slice(q * m_shard // 2, (q + 1) * m_shard // 2)
                        nc.sync.dma_start(fold_out[:, sl], fold[:, sl])
                nc.vector.max(out=fv_sb[:, n * 8:(n + 1) * 8],
                              in_=src[:])
                # stream forward outputs out as they complete; the final
                # chunk's 8-wide slice goes alone so the earlier bulk is
                # not gated on the last DVE op
                if (n + 1) % (n_chunks // 4) == 0 and not last:
                    sl = slice((n + 1 - n_chunks // 4) * 8, (n + 1) * 8)
                    nc.sync.dma_start(fwd_val[:, sl], fv_sb[:, sl])
                elif n == n_chunks - 2:
                    sl = slice((n_chunks - n_chunks // 4) * 8, (n + 1) * 8)
                    nc.sync.dma_start(fwd_val[:, sl], fv_sb[:, sl])
            sl = slice((n_chunks - 1) * 8, n_chunks * 8)
            nc.sync.dma_start(fwd_val[:, sl], fv_sb[:, sl])

    nc.compile()
    return nc


_KERNEL_CACHE = {}


def get_kernel(dtype_mode=DTYPE_MODE):
    if dtype_mode not in _KERNEL_CACHE:
        _KERNEL_CACHE[dtype_mode] = build_kernel(dtype_mode=dtype_mode)
    return _KERNEL_CACHE[dtype_mode]


# --------------------------------------------------------------------------
# Host side
# --------------------------------------------------------------------------

def _decode_top8(arr, chunks):
    """[128, chunks*8] -> [chunks*128, 8] with row r = chunk*128 + partition."""
    return arr.reshape(128, chunks, 8).transpose(1, 0, 2).reshape(chunks * 128, 8)


def run_device(descriptors0, descriptors1, dtype_mode=DTYPE_MODE):
    """Run the SPMD kernel on 8 cores. Returns per-core raw outputs."""
    from concourse.bass_utils import run_bass_kernel_spmd

    nc = get_kernel(dtype_mode)
    d0 = np.ascontiguousarray(descriptors0[0]).astype(np.float32, copy=False)
    d1 = np.ascontiguousarray(descriptors1[0]).astype(np.float32, copy=False)
    kf = F_DIM // 128

    def shard(a, c):
        return np.ascontiguousarray(
            a[:, c * M_SHARD:(c + 1) * M_SHARD]).reshape(kf, 128, M_SHARD)

    if dtype_mode == "fp32":
        d0r = d0.reshape(kf, 128, N_KPTS)
        in_maps = [{"d0": d0r, "d1": shard(d1, c)} for c in range(N_CORES)]
    else:
        h0, h0s, l0s = _split_f16(d0)
        h1, h1s, l1s = _split_f16(d1)
        d0m = {"d0h": h0.reshape(kf, 128, N_KPTS),
               "d0hs": h0s.reshape(kf, 128, N_KPTS),
               "d0ls": l0s.reshape(kf, 128, N_KPTS)}
        in_maps = [dict(d0m, d1h=shard(h1, c), d1hs=shard(h1s, c),
                        d1ls=shard(l1s, c)) for c in range(N_CORES)]
    last_err = None
    for _attempt in range(3):
        try:
            res = run_bass_kernel_spmd(nc, in_maps, list(range(N_CORES)))
            # materialize device arrays now so transient device failures
            # surface inside this retry loop, not later in postprocess
            return [{k: np.asarray(v) for k, v in r.items()}
                    for r in res.results]
        except Exception as e:  # rare transient device-unrecoverable flakes
            last_err = e
    raise last_err


def postprocess(results):
    """Merge per-core device outputs into the reference's 4 output arrays."""
    n = N_KPTS
    n_chunks = n // 128
    m_chunks = M_SHARD // 128

    # ---- forward: merge per-core top-2 into global top-2 ----
    m1 = np.empty((N_CORES, n), np.float32)
    m2 = np.empty((N_CORES, n), np.float32)
    for c in range(N_CORES):
        vals = _decode_top8(results[c]["fwd_val"], n_chunks)
        m1[c] = vals[:, 0]
        m2[c] = vals[:, 1]

    w = np.argmax(m1, axis=0)                      # first max on ties
    rows = np.arange(n)
    s1 = m1[w, rows]
    m1_masked = m1.copy()
    m1_masked[w, rows] = -np.inf
    s2 = np.maximum(m1_masked.max(axis=0), m2[w, rows]).astype(np.float32)

    # ---- backward: reduce the per-lane fold over its 128 lanes ----
    cm1 = np.empty(M_KPTS, np.float32)
    cm2 = np.empty(M_KPTS, np.float32)
    lane = np.empty(M_KPTS, np.int64)
    for c in range(N_CORES):
        f = results[c]["fold"]                     # [128, M_SHARD]
        sl = slice(c * M_SHARD, (c + 1) * M_SHARD)
        lane[sl] = np.argmax(f, axis=0)
        cm1[sl] = f[lane[sl], np.arange(M_SHARD)]
        cm2[sl] = np.partition(f, 126, axis=0)[126]

    # ---- exact reference arithmetic (float32) ----
    def dist(s):
        return SQRT_2 * np.sqrt(np.maximum(ONE - s, CLIP_LO))

    fd1, fd2 = dist(s1), dist(s2)
    fwd_ok = (fd1 / fd2) < np.float32(1.0)
    bd1, bd2 = dist(cm1), dist(cm2)
    bck_ok = (bd1 / bd2) < np.float32(1.0)

    # ---- mutual NN + index recovery by exact value matching ----
    # Row i matches column j iff s1[i] == cm1[j] (bitwise: both sides are
    # the same f32 S element).  Search row i's winning core's 1024 column
    # maxes for the value s1[i].  A hit from a DIFFERENT dot product that
    # happens to collide bitwise is filtered by the argmax-lane check
    # (i % 128 must equal the column's winning lane); residual ambiguity
    # is reported (expected never).
    fwd_nn = np.zeros(n, np.int64)
    found = np.zeros(n, bool)
    ambig = 0
    for c in range(N_CORES):
        rows_c = np.nonzero(w == c)[0]
        if rows_c.size == 0:
            continue
        M1c = cm1[c * M_SHARD:(c + 1) * M_SHARD]
        order = np.argsort(M1c, kind="stable")
        sv = M1c[order]
        tgt = s1[rows_c]
        lo = np.searchsorted(sv, tgt, side="left")
        hi = np.searchsorted(sv, tgt, side="right")
        lane_c = lane[c * M_SHARD:(c + 1) * M_SHARD]
        for ri, l, h in zip(rows_c, lo, hi):
            if h == l:
                continue
            cands = [j for j in order[l:h] if lane_c[j] == ri % 128]
            if len(cands) == 1:
                fwd_nn[ri] = cands[0] + c * M_SHARD
                found[ri] = True
            elif len(cands) > 1:
                ambig += 1
    if ambig:
        print(f"WARNING: {ambig} ambiguous column-max value collisions")

    mutual = fwd_ok & found & bck_ok[fwd_nn]

    indices0 = np.where(mutual, fwd_nn, -1)[None, :].astype(np.int32)
    mscores0 = (indices0 > 0).astype(np.int32)
    matches1 = np.full((1, M_KPTS), -1, dtype=np.int32)
    mscores1 = np.zeros((1, M_KPTS), dtype=np.float32)
    return indices0, matches1, mscores0, mscores1


def kernel(descriptors0, descriptors1, keypoints0, keypoints1):
    results = run_device(descriptors0, descriptors1)
    return postprocess(results)
